# revision 26
# baseline (speedup 1.0000x reference)
"""Trainium2 Bass kernel for nn_Cross_Attention_27178553049599 (v7).

Reference computation (per batch sample b):
    q = x @ Wq ; k = y @ Wk ; v = x @ Wv
    attn = softmax(q @ k^T * SCALE); attn = where(attn < 0.6, 0, attn)
    out  = (attn @ v) @ Wp + bp

Facts exploited:
  * softmax rows sum to 1 => at most ONE entry per row survives the 0.6
    threshold; out_row = p * (v @ Wp)[m*] + bp, p = exp(s*)/Z.
  * v @ Wp = x @ (Wv @ Wp);  q @ k^T = x @ (Wq @ Wk^T) @ y^T.

v7 notes:
  * y^T / x^T prepared host-side (layout + fp16 hi/lo split of inputs),
    DMA'd straight to SBUF.
  * hot loop per row-block: 16 fp16 matmuls -> 2 exp-activations with
    Z-accumulate -> ONE tensor_scalar (expS >= 0.59*Z) with count
    accumulate (fast DVE mode).  No index scan in the loop: on trn2
    STT runs at 1x only (measured 2.35us/2048) and was the v6
    bottleneck; tensor_scalar is the only multi-mode op.
  * flagged rows (count>0) are compacted per 8-rb group via
    sparse_gather; the group recomputes its rows' S exactly (fp16
    hi/lo 3-term), gets Z/max from the activation, and finds the
    survivor index m* by scanning the recomputed expR (two all-fp16
    half-scans, iota 1..2048 exact) -> gather x[m*] -> vpr -> scatter.
  * group-0 chain is spread over rb8..rb14 so the in-order engine
    queues never stall the loop; group 1 is the tail.
"""

import numpy as np

import concourse.bass as bass
import concourse.mybir as mybir
import concourse.tile as tile
from concourse.bass import IndirectOffsetOnAxis

F32 = mybir.dt.float32
F16 = mybir.dt.float16
I32 = mybir.dt.int32
U32 = mybir.dt.uint32
ALU = mybir.AluOpType
EXP = mybir.ActivationFunctionType.Exp

P = 128
B, N, D = 4, 4096, 256
NH = 2048
SCALE = (D // 8) ** -0.5
THRESH = 0.6
LO = 0.59
EXP_BIAS = -14.0
NCORES = 8
RBLK = NH // P


def _build_program() -> bass.Bass:
    import concourse.bacc as bacc

    nc = bacc.Bacc("TRN2", target_bir_lowering=False, debug=False)

    xq = nc.dram_tensor("xq", [NH, D], F32, kind="ExternalInput").ap()
    x = nc.dram_tensor("x", [N, D], F32, kind="ExternalInput").ap()
    w_in = {
        w: nc.dram_tensor(w, [D, D], F32, kind="ExternalInput").ap()
        for w in ("Wq", "Wk", "Wv", "Wp")
    }
    bp = nc.dram_tensor("bp", [D], F32, kind="ExternalInput").ap()
    yThi_in = nc.dram_tensor("c_yThi", [P, 2, N], F16, kind="ExternalInput").ap()
    yTlo_in = nc.dram_tensor("c_yTlo", [P, 2, N], F16, kind="ExternalInput").ap()
    xTh_in = nc.dram_tensor("c_xTh", [P, 2, NH], F16, kind="ExternalInput").ap()
    ident_in = nc.dram_tensor("c_ident", [P, P], F32, kind="ExternalInput").ap()
    iotaA_in = nc.dram_tensor("c_iotaA", [NH], F16, kind="ExternalInput").ap()
    iotaB_in = nc.dram_tensor("c_iotaB", [NH], F16, kind="ExternalInput").ap()
    idp1_in = nc.dram_tensor("c_idp1", [P, RBLK], F32, kind="ExternalInput").ap()

    out = nc.dram_tensor("out", [NH, D], F32, kind="ExternalOutput").ap()

    with tile.TileContext(nc) as tc:
        _body(tc, xq, x, w_in, bp, yThi_in, yTlo_in, xTh_in,
              ident_in, iotaA_in, iotaB_in, idp1_in, out)
    nc.compile()
    return nc


def _body(tc, xq, x, w_in, bp, yThi_in, yTlo_in, xTh_in,
          ident_in, iotaA_in, iotaB_in, idp1_in, out):
    from contextlib import ExitStack
    from concourse import library_config
    from concourse.tile import add_dep_helper

    nc = tc.nc
    with ExitStack() as ctx:
        const = ctx.enter_context(tc.tile_pool(name="const", bufs=1))
        big = ctx.enter_context(tc.tile_pool(name="big", bufs=1))
        small = ctx.enter_context(tc.tile_pool(name="small", bufs=1))

        # ---- sync queue, strict priority order: ramp-critical first ----
        ident = const.tile([P, P], F32)
        nc.sync.dma_start(out=ident, in_=ident_in)
        w_sb = {}
        for wname, wap in w_in.items():
            wt = const.tile([P, 2, D], F32, name=f"w_{wname}")
            nc.sync.dma_start(out=wt, in_=wap.rearrange("(a p) e -> p a e", p=P))
            w_sb[wname] = wt
        xTh = big.tile([P, 2, NH], F16)
        nc.sync.dma_start(out=xTh, in_=xTh_in)
        yThi = big.tile([P, 2, N], F16)
        nc.sync.dma_start(out=yThi, in_=yThi_in)
        # -- not ramp-critical below --
        bp_t = const.tile([P, D], F32)
        nc.sync.dma_start(
            out=bp_t,
            in_=bass.AP(tensor=bp.tensor, offset=bp.offset, ap=[[0, P], [1, D]]),
        )
        yTlo = big.tile([P, 2, N], F16)        # only needed by srep (late)
        nc.sync.dma_start(out=yTlo, in_=yTlo_in)
        iotaA = const.tile([P, NH], F16)
        nc.sync.dma_start(
            out=iotaA,
            in_=bass.AP(tensor=iotaA_in.tensor, offset=iotaA_in.offset,
                        ap=[[0, P], [1, NH]]),
        )
        iotaB = const.tile([P, NH], F16)
        nc.sync.dma_start(
            out=iotaB,
            in_=bass.AP(tensor=iotaB_in.tensor, offset=iotaB_in.offset,
                        ap=[[0, P], [1, NH]]),
        )
        idp1 = const.tile([P, RBLK], F32)      # natural row id + 1
        nc.sync.dma_start(out=idp1, in_=idp1_in)

        exp_bias = const.tile([P, 1], F32)
        nc.vector.memset(exp_bias, EXP_BIAS)

        qTp = big.tile([P, 2, NH], F16)
        Wqk = const.tile([P, 2, D], F32)
        Wqk_h = const.tile([P, 2, D], F16)
        Wvp_h = const.tile([P, 2, D], F16)

        with tc.tile_pool(name="pro_ps", bufs=6, space="PSUM") as pro:
            wT = {}
            for wname in ("Wq", "Wk", "Wv"):
                t = const.tile([P, 2, D], F32, name=f"wT_{wname}")
                for a in range(2):
                    pt = pro.tile([P, 512], F32, tag="pro")
                    for b_ in range(2):
                        nc.tensor.transpose(
                            out=pt[:, b_ * P:(b_ + 1) * P],
                            in_=w_sb[wname][:, b_, a * P:(a + 1) * P],
                            identity=ident,
                        )
                    nc.vector.tensor_copy(t[:, a, :], pt[:, :D])
                wT[wname] = t

            for a in range(2):
                pq = pro.tile([P, 512], F32, tag="pro")
                for cb in range(2):
                    nc.tensor.matmul(
                        out=pq[:, :D],
                        lhsT=wT["Wq"][:, cb, a * P:(a + 1) * P],
                        rhs=wT["Wk"][:, cb, :],
                        start=cb == 0, stop=cb == 1,
                    )
                nc.vector.tensor_copy(Wqk[:, a, :], pq[:, :D])
                nc.vector.tensor_copy(Wqk_h[:, a, :], pq[:, :D])

            for a in range(2):
                pv = pro.tile([P, 512], F32, tag="pro")
                for eb in range(2):
                    nc.tensor.matmul(
                        out=pv[:, :D],
                        lhsT=wT["Wv"][:, eb, a * P:(a + 1) * P],
                        rhs=w_sb["Wp"][:, eb, :],
                        start=eb == 0, stop=eb == 1,
                    )
                nc.vector.tensor_copy(Wvp_h[:, a, :], pv[:, :D])

            # qT = (xq @ Wqk)^T = Wqk^T @ xq^T   [d, rows] fp16
            # nt outer so rb0's columns (both a halves) are ready first
            for nt in range(NH // 512):
                for a in range(2):
                    ps = pro.tile([P, 512], F32, tag="pro")
                    for kb in range(2):
                        nc.tensor.matmul(
                            out=ps,
                            lhsT=Wqk_h[:, kb, a * P:(a + 1) * P],
                            rhs=xTh[:, kb, nt * 512:(nt + 1) * 512],
                            start=kb == 0, stop=kb == 1,
                        )
                    if a == 0:
                        nc.scalar.copy(qTp[:, a, nt * 512:(nt + 1) * 512], ps)
                    else:
                        nc.vector.tensor_copy(
                            qTp[:, a, nt * 512:(nt + 1) * 512], ps)

        lib_inst = nc.gpsimd.load_library(library_config.sparse_gather)
        comp_w = small.tile([16, 8], F32, name="comp_w")
        nfound_w = small.tile([1, 1], U32, name="nf_w")
        sg_w = nc.gpsimd.sparse_gather(out=comp_w, in_=ident[0:16, 0:64],
                                       num_found=nfound_w)
        add_dep_helper(sg_w.ins, lib_inst.ins, reason="sg ucode warmup")
        lib_state = {"lib": library_config.sparse_gather, "ins": lib_inst}

        def want_lib(lib):
            if lib_state["lib"] is not lib:
                lib_state["ins"] = nc.gpsimd.load_library(lib)
                lib_state["lib"] = lib
            return lib_state["ins"]

        sel_cols = small.tile([P, RBLK], F32)
        cnt_all = small.tile([P, RBLK], F32)
        grp = [dict(), dict()]
        expR = [big.tile([P, N], F16, name=f"expR{g}") for g in range(2)]

        with tc.tile_pool(name="S_ps", bufs=2, space="PSUM") as sps, \
             tc.tile_pool(name="expS_p", bufs=3) as expp, \
             tc.tile_pool(name="junk_p", bufs=3) as junkp, \
             tc.tile_pool(name="sm", bufs=6) as sm:

            def emit_rb_mm(rb, q):
                sp = sps.tile([P, NH], F32, tag="S")
                for mt in range(4):
                    for a in range(2):
                        nc.tensor.matmul(
                            out=sp[:, mt * 512:(mt + 1) * 512],
                            lhsT=qTp[:, a, rb * P:(rb + 1) * P],
                            rhs=yThi[:, a, q * NH + mt * 512:
                                     q * NH + (mt + 1) * 512],
                            start=a == 0, stop=a == 1,
                        )
                return sp

            def emit_rb_rest(rb, halves):
                expS = expp.tile([P, N], F16)
                zp = sm.tile([P, 2], F32)
                for q in range(2):
                    nc.scalar.activation(
                        out=expS[:, q * NH:(q + 1) * NH],
                        in_=halves[q],
                        func=EXP, scale=SCALE, bias=exp_bias,
                        accum_out=zp[:, q:q + 1],
                    )
                thr0 = sm.tile([P, 1], F32)
                nc.vector.tensor_scalar_mul(thr0, zp[:, 0:1], LO)
                thr = sm.tile([P, 1], F32)
                nc.vector.scalar_tensor_tensor(
                    out=thr, in0=zp[:, 1:2], scalar=LO, in1=thr0,
                    op0=ALU.mult, op1=ALU.add,
                )
                junk = junkp.tile([P, N], F16, tag="junk")
                nc.vector.tensor_scalar(
                    junk, expS, thr, scalar2=0.0,
                    op0=ALU.is_ge, op1=ALU.add,
                    accum_out=cnt_all[:, rb:rb + 1],
                )

            def emit_rb(rb):
                emit_rb_rest(rb, [emit_rb_mm(rb, 0), emit_rb_mm(rb, 1)])

            def sel_part(cols):
                # sel = flag*(row+1) - 1  (idp1 holds row+1)
                csl = slice(cols.start, cols.stop)
                n = cols.stop - cols.start
                flag = sm.tile([P, 8], F32, tag="flag")
                nc.vector.tensor_scalar(flag[:, :n], cnt_all[:, csl], 0.0,
                                        scalar2=None, op0=ALU.is_gt)
                nc.vector.tensor_tensor(flag[:, :n], flag[:, :n],
                                        idp1[:, csl], op=ALU.mult)
                nc.vector.tensor_scalar(sel_cols[:, csl], flag[:, :n], -1.0,
                                        scalar2=None, op0=ALU.add)

            def head_a(g):
                sel16 = small.tile([16, 64], F32, name=f"sel16_{g}")
                nc.sync.dma_start(out=sel16, in_=sel_cols[:, g * 8:(g + 1) * 8])
                comp = small.tile([16, 8], F32, name=f"comp_{g}")
                nc.vector.memset(comp, -7.0)
                nfound = small.tile([1, 1], U32, name=f"nf_{g}")
                sg_lib = want_lib(library_config.sparse_gather)
                sg = nc.gpsimd.sparse_gather(out=comp, in_=sel16,
                                             num_found=nfound)
                add_dep_helper(sg.ins, sg_lib.ins, reason="sg ucode")
                grp[g]["comp"] = comp

            def head_b(g):
                st = grp[g]
                ids_f = small.tile([P, 1], F32, name=f"idsf_{g}")
                nc.sync.dma_start(out=ids_f, in_=st["comp"])
                ids = small.tile([P, 1], I32, name=f"ids_{g}")
                nc.vector.tensor_copy(ids, ids_f)
                nc.vector.tensor_scalar(ids, ids, 0, scalar2=None, op0=ALU.max)
                nc.vector.tensor_scalar(ids, ids, NH - 1, scalar2=None,
                                        op0=ALU.min)
                st["ids"] = ids

            def head_c(g):
                st = grp[g]
                xr = small.tile([P, D], F32, name=f"xr_{g}")
                nc.gpsimd.indirect_dma_start(
                    out=xr, out_offset=None, in_=xq,
                    in_offset=IndirectOffsetOnAxis(ap=st["ids"], axis=0),
                    bounds_check=NH - 1, oob_is_err=False,
                )
                st["xr"] = xr

            def rep_u(g):
                st = grp[g]
                cpy = (nc.vector.tensor_copy if g == 0 else nc.scalar.copy)
                spt = sps.tile([P, NH], F32, tag="S")
                xrT = small.tile([P, 2, P], F32, name=f"xrT_{g}")
                for kb in range(2):
                    sl = spt[:, kb * P:(kb + 1) * P]
                    nc.tensor.transpose(out=sl,
                                        in_=st["xr"][:, kb * P:(kb + 1) * P],
                                        identity=ident)
                    cpy(xrT[:, kb, :], sl)
                uhT = small.tile([P, 2, P], F16, name=f"uhT_{g}")
                ulT = small.tile([P, 2, P], F16, name=f"ulT_{g}")
                for a in range(2):
                    sl = spt[:, (4 + a) * P:(5 + a) * P]
                    for kb in range(2):
                        nc.tensor.matmul(
                            out=sl,
                            lhsT=Wqk[:, kb, a * P:(a + 1) * P],
                            rhs=xrT[:, kb, :],
                            start=kb == 0, stop=kb == 1,
                        )
                    cpy(uhT[:, a, :], sl)
                    nc.vector.scalar_tensor_tensor(
                        out=ulT[:, a, :], in0=uhT[:, a, :], scalar=-1.0,
                        in1=sl, op0=ALU.mult, op1=ALU.add,
                    )
                st["uhT"], st["ulT"] = uhT, ulT

            def rep_srep(g, half):
                st = grp[g]
                uhT, ulT = st["uhT"], st["ulT"]
                srp = sps.tile([P, NH], F32, tag="S")
                for mt in range(4):
                    combos = [(uhT, yThi), (uhT, yTlo), (ulT, yThi)]
                    i_mm, n_mm = 0, 6
                    for (wt_, yt_) in combos:
                        for a in range(2):
                            nc.tensor.matmul(
                                out=srp[:, mt * 512:(mt + 1) * 512],
                                lhsT=wt_[:, a, :],
                                rhs=yt_[:, a, half * NH + mt * 512:
                                        half * NH + (mt + 1) * 512],
                                start=i_mm == 0, stop=i_mm == n_mm - 1,
                            )
                            i_mm += 1
                if half == 0:
                    st["zpR"] = small.tile([P, 2], F32, name=f"zpR_{g}")
                    st["mxR"] = small.tile([P, 2], F32, name=f"mxR_{g}")
                nc.scalar.activation(
                    out=expR[g][:, half * NH:(half + 1) * NH],
                    in_=srp, func=EXP, scale=SCALE, bias=exp_bias,
                    accum_out=st["zpR"][:, half:half + 1],
                )
                # survivor value = row max (single survivor possible)
                nc.vector.tensor_reduce(
                    st["mxR"][:, half:half + 1],
                    expR[g][:, half * NH:(half + 1) * NH],
                    axis=mybir.AxisListType.X, op=ALU.max,
                )

            def rep_scan(g):
                st = grp[g]
                zR = small.tile([P, 1], F32, name=f"zR_{g}")
                nc.vector.tensor_add(zR, st["zpR"][:, 0:1], st["zpR"][:, 1:2])
                st["zR"] = zR
                thrR = small.tile([P, 1], F32, name=f"thrR_{g}")
                nc.vector.tensor_scalar_mul(thrR, zR, LO)
                jx = small.tile([P, 2], F32, name=f"jx_{g}")
                for q in range(2):
                    junk = junkp.tile([P, NH], F16, tag="junk")
                    eng = nc.vector
                    eng.scalar_tensor_tensor(
                        out=junk,
                        in0=expR[g][:, q * NH:(q + 1) * NH],
                        scalar=thrR,
                        in1=(iotaA if q == 0 else iotaB),
                        op0=ALU.is_ge, op1=ALU.mult,
                        accum_out=jx[:, q:q + 1],
                    )
                cB = small.tile([P, 1], F32, name=f"cB_{g}")
                nc.vector.tensor_scalar(cB, jx[:, 1:2], 0.0,
                                        scalar2=None, op0=ALU.is_gt)
                ji_f = small.tile([P, 1], F32, name=f"jif_{g}")
                nc.vector.tensor_add(ji_f, jx[:, 0:1], jx[:, 1:2])
                nc.vector.scalar_tensor_tensor(
                    out=ji_f, in0=cB, scalar=float(NH), in1=ji_f,
                    op0=ALU.mult, op1=ALU.add,
                )
                nc.vector.tensor_scalar(ji_f, ji_f, -1.0, scalar2=None,
                                        op0=ALU.add)
                ji = small.tile([P, 1], I32, name=f"ji_{g}")
                nc.vector.tensor_copy(ji, ji_f)
                nc.vector.tensor_scalar(ji, ji, 0, scalar2=None, op0=ALU.max)
                nc.vector.tensor_scalar(ji, ji, N - 1, scalar2=None, op0=ALU.min)
                st["ji"] = ji
                xm = small.tile([P, D], F32, name=f"xm_{g}")
                nc.gpsimd.indirect_dma_start(
                    out=xm, out_offset=None, in_=x,
                    in_offset=IndirectOffsetOnAxis(ap=ji, axis=0),
                    bounds_check=N - 1, oob_is_err=False,
                )
                st["xm"] = xm

            def rep_vpr(g):
                st = grp[g]
                cpy = (nc.vector.tensor_copy if g == 0 else nc.scalar.copy)
                spt = sps.tile([P, NH], F32, tag="S")
                xmh = small.tile([P, 2, P], F16, name=f"xmh_{g}")
                for kb in range(2):
                    sl = spt[:, kb * P:(kb + 1) * P]
                    nc.tensor.transpose(out=sl,
                                        in_=st["xm"][:, kb * P:(kb + 1) * P],
                                        identity=ident)
                    cpy(xmh[:, kb, :], sl)
                for kb in range(2):
                    nc.tensor.matmul(
                        out=spt[:, 1024:1024 + D],
                        lhsT=xmh[:, kb, :],
                        rhs=Wvp_h[:, kb, :],
                        start=kb == 0, stop=kb == 1,
                    )
                vpr = small.tile([P, D], F32, name=f"vpr_{g}")
                cpy(vpr, spt[:, 1024:1024 + D])
                st["vpr"] = vpr

            def rep_finish(g):
                st = grp[g]
                zR = st["zR"]
                mx = small.tile([P, 1], F32, name=f"mx_{g}")
                nc.vector.tensor_tensor(mx, st["mxR"][:, 0:1],
                                        st["mxR"][:, 1:2], op=ALU.max)
                thrT = small.tile([P, 1], F32, name=f"thrT_{g}")
                nc.vector.tensor_scalar_mul(thrT, zR, THRESH)
                dec = small.tile([P, 1], F32, name=f"dec_{g}")
                nc.vector.scalar_tensor_tensor(
                    out=dec, in0=mx, scalar=thrT, in1=mx,
                    op0=ALU.is_ge, op1=ALU.mult,
                )
                rzR = small.tile([P, 1], F32, name=f"rzR_{g}")
                nc.vector.reciprocal(rzR, zR)
                gR = small.tile([P, 1], F32, name=f"gR_{g}")
                nc.vector.tensor_tensor(gR, dec, rzR, op=ALU.mult)
                outR = small.tile([P, D], F32, name=f"outR_{g}")
                nc.vector.scalar_tensor_tensor(
                    out=outR, in0=st["vpr"], scalar=gR, in1=bp_t,
                    op0=ALU.mult, op1=ALU.add,
                )
                nc.gpsimd.indirect_dma_start(
                    out=out, out_offset=IndirectOffsetOnAxis(ap=st["ids"], axis=0),
                    in_=outR, in_offset=None,
                    bounds_check=NH - 1, oob_is_err=False,
                )

            for rb in range(RBLK):
                emit_rb(rb)
                if rb == 1:
                    # bp prefill on the (now idle) sync queue
                    for rbg in range(4):
                        dst = bass.AP(
                            tensor=out.tensor, offset=out.offset + rbg * 4 * P * D,
                            ap=[[D, P], [P * D, 4], [1, D]],
                        )
                        src = bass.AP(tensor=bp_t.tensor, offset=bp_t.offset,
                                      ap=[bp_t.ap[0], [0, 4], [1, D]])
                        nc.sync.dma_start(out=dst, in_=src)
                elif rb == 8:
                    sel_part(slice(0, 8))
                    head_a(0)
                elif rb == 9:
                    head_b(0)
                elif rb == 10:
                    head_c(0)
                elif rb == 11:
                    rep_u(0)
                elif rb == 12:
                    rep_srep(0, 0)
                elif rb == 14:
                    rep_srep(0, 1)
            # ---- tail: group 1 (launch first, then finish group 0) ----
            sel_part(slice(8, 16))
            head_a(1)
            rep_scan(0)
            head_b(1)
            rep_vpr(0)
            rep_finish(0)
            head_c(1)
            rep_u(1)
            rep_srep(1, 0)
            rep_srep(1, 1)
            rep_scan(1)
            rep_vpr(1)
            rep_finish(1)


_NC_CACHE = None


def _get_program():
    global _NC_CACHE
    if _NC_CACHE is None:
        _NC_CACHE = _build_program()
    return _NC_CACHE


def _perm_ids(n):
    """row/key id for packed-transpose column order: col -> 512c+4p+j."""
    cols = np.arange(n)
    c, r = cols // 512, cols % 512
    j, p = r // P, r % P
    return 512 * c + 4 * p + j


def _make_in_maps(x, y, Wq, Wk, Wv, Wp, bp):
    f32, f16 = np.float32, np.float16
    x = np.asarray(x, f32)
    y = np.asarray(y, f32)
    perm_m = _perm_ids(N)
    perm_r = _perm_ids(NH)
    iotaA = (1 + perm_m[:NH]).astype(f16)
    iotaB = (1 + perm_m[NH:] - NH).astype(f16)
    consts = {
        "Wq": np.ascontiguousarray(Wq, f32),
        "Wk": np.ascontiguousarray(Wk, f32),
        "Wv": np.ascontiguousarray(Wv, f32),
        "Wp": np.ascontiguousarray(Wp, f32),
        "bp": np.ascontiguousarray(bp, f32),
        "c_ident": np.eye(P, dtype=f32),
        "c_iotaA": iotaA,
        "c_iotaB": iotaB,
        "c_idp1": (1.0 + perm_r.reshape(RBLK, P).T).astype(f32),
    }
    in_maps = []
    for core in range(NCORES):
        b, half = core // 2, core % 2
        xb = np.ascontiguousarray(x[b])
        xqb = np.ascontiguousarray(xb[half * NH:(half + 1) * NH])
        yT = np.ascontiguousarray(y[b][perm_m].T)          # [D, N]
        yhi = yT.astype(f16)
        ylo = (yT - yhi.astype(f32)).astype(f16)
        xT = np.ascontiguousarray(xqb[perm_r].T)           # [D, NH]
        xhi = xT.astype(f16)
        in_maps.append({
            "x": xb,
            "xq": xqb,
            "c_yThi": np.ascontiguousarray(
                yhi.reshape(2, P, N).transpose(1, 0, 2)),
            "c_yTlo": np.ascontiguousarray(
                ylo.reshape(2, P, N).transpose(1, 0, 2)),
            "c_xTh": np.ascontiguousarray(
                xhi.reshape(2, P, NH).transpose(1, 0, 2)),
            **consts,
        })
    return in_maps


def kernel(x, y, Wq, Wk, Wv, Wp, bp):
    from concourse.bass_utils import run_bass_kernel_spmd

    nc = _get_program()
    in_maps = _make_in_maps(x, y, Wq, Wk, Wv, Wp, bp)
    res = run_bass_kernel_spmd(nc, in_maps, list(range(NCORES)))
    outv = np.empty((B, N, D), np.float32)
    for core in range(NCORES):
        b, half = core // 2, core % 2
        outv[b, half * NH:(half + 1) * NH] = res.results[core]["out"]
    return outv
